# revision 3
# baseline (speedup 1.0000x reference)
"""Luong attention kernel v5 — optimized for the CoreSim cost model.

Cost-model facts (probed):
  - 3 overlapping DMA queues (sync/SP, scalar/ACT, gpsimd/Pool); charge =
    OUT per-partition bytes x 0.3855 ns -> casting f32->fp16 DMA on gpsimd
    is charged half. BUT: SBUF transpose-DMAs serialize pairwise against
    SWDGE (Pool) DMAs in the Tile scheduler (HW deadlock guard), so a
    transpose-based score path cannot overlap the Pool stream — no
    transposes anywhere in this kernel.
  - matmul charge = out-free-size x cycles_per_row; LoadStationary free.
    "Flipped" matmuls (lhsT = [128,X] stationary, out = [X,1]) are ~free
    and pstate-immune -> context is nearly free on PE.
  - fused dot products: DVE scalar_tensor_tensor ~2.5us/[128,2048] tile;
    DVE tensor_mul (fp16) ~1.43us; gpsimd tensor_mul ~2.33us (Pool engine
    overlaps its own DMA queue); ACT Copy-with-accum reduce ~2.43us.
  - gpsimd partition_all_reduce / partition_broadcast (attn library) run
    on the Pool engine.

Structure (per core: BL=8 batches, 16 s-tiles [128s, 2048h] each):
  - 12 tiles/batch stream as fp16 casting-DMAs on gpsimd (2-tile chunks);
    4 tiles/batch stream as f32 on sync.
  - scores: per-tile path mix balancing DVE / ACT / Pool engines:
      F: f32 tiles, DVE fused mul+accum (vs broadcast fp16 q)
      X: fp16 tiles, DVE fused mul+accum
      Y: fp16 tiles, DVE products + ACT in-place copy-accum reduce
      G: fp16 tiles, gpsimd products + ACT in-place copy-accum reduce
  - per-batch A/B half-split softmax with provisional max and exact
    alpha-combine so half-A tiles release early (ring pressure).
  - context: flipped PE matmuls, pcx[p, hb] = sum_s E[s,128hb+p] w[s];
    host assemble un-transposes [P,16] -> [H].
"""

import numpy as np

B, S, H = 64, 2048, 2048
NCORES = 8
BL = B // NCORES          # local batches per core
P = 128                   # SBUF partitions
NT = S // P               # 16 s-tiles per batch
NHB = H // P              # 16 h-blocks
HALF = NT // 2
UCH = 2                   # s-tiles per DMA chunk

# score-path assignment per tile index (halves: A = 0..7, B = 8..15)
G_TS = []                              # (gpsimd ucode ops rejected by walrus)
Y_TS = [0, 1, 2, 3, 8, 9, 10, 11, 12, 13]  # DVE products + ACT reduce
X_TS = [4, 5]                          # DVE fused stt
F_TS = [6, 7, 14, 15]                  # f32 tiles, DVE fused stt (sync queue)
A_TILES = list(range(HALF))
B_TILES = list(range(HALF, NT))
F16_TS = sorted(set(range(NT)) - set(F_TS))

_NC = None


def _build_nc():
    import concourse.bass as bass
    import concourse.tile as tile
    from concourse import mybir
    from concourse import bass_isa, library_config

    F32 = mybir.dt.float32
    F16 = mybir.dt.float16
    Alu = mybir.AluOpType
    Act = mybir.ActivationFunctionType
    Red = bass_isa.ReduceOp

    nc = bass.Bass()
    hid = nc.declare_dram_parameter("hidden", [BL, H], F32, isOutput=False)
    enc = nc.declare_dram_parameter("enc", [BL, S, H], F32, isOutput=False)
    ctx_out = nc.declare_dram_parameter("ctx_out", [BL, P, NHB], F32, isOutput=True)
    attn_out = nc.declare_dram_parameter("attn_out", [BL, P, NT], F32, isOutput=True)

    with tile.TileContext(nc) as tc:
        with (
            tc.tile_pool(name="hpool", bufs=13) as hpool,    # fp16 s-tile chunks
            tc.tile_pool(name="fpool", bufs=4) as fpool,     # f32 s-tile chunks
            tc.tile_pool(name="qpool", bufs=2) as qpool,
            tc.tile_pool(name="tmpp", bufs=4) as tmpp,
            tc.tile_pool(name="smallp", bufs=2) as smallp,
            tc.tile_pool(name="consts", bufs=1) as consts,
            tc.tile_pool(name="pqb", bufs=1, space="PSUM") as pqb,
            tc.tile_pool(name="psml", bufs=2, space="PSUM") as psml,
            tc.tile_pool(name="pctxa", bufs=2, space="PSUM") as pctxa,
            tc.tile_pool(name="pctxb", bufs=2, space="PSUM") as pctxb,
        ):
            neg_ones_row = consts.tile([1, P], F32, tag="negones")
            nc.vector.memset(neg_ones_row, -1.0)
            ones_row32 = consts.tile([1, P], F32, tag="ones32")
            nc.vector.memset(ones_row32, 1.0)
            ones_row = consts.tile([1, P], F16, tag="ones16")
            nc.vector.memset(ones_row, 1.0)

            deferred = []

            def flush_deferred():
                for fn in deferred:
                    fn()
                deferred.clear()

            def qprep(bi):
                # fp16 q row straight from HBM via casting DMA ([1, N] DMAs
                # are charged on total bytes; fp16 out halves it)
                q16r = qpool.tile([1, H], F16, tag="q16r", name=f"q16r_{bi}")
                nc.gpsimd.dma_start(out=q16r, in_=hid[bi : bi + 1, :])
                qb = qpool.tile([P, H], F16, tag="qb", name=f"qb_{bi}")
                psq = pqb.tile([P, 1024], F32, tag="pqb", name=f"pqb_{bi}")
                for j in range(2):
                    for k in range(2):
                        sl = slice((2 * j + k) * 512, (2 * j + k + 1) * 512)
                        nc.tensor.matmul(
                            psq[:, k * 512 : (k + 1) * 512],
                            lhsT=ones_row,
                            rhs=q16r[:, sl],
                            start=True,
                            stop=True,
                        )
                    if j == 0:
                        nc.vector.tensor_copy(
                            out=qb[:, j * 1024 : (j + 1) * 1024], in_=psq
                        )
                    else:
                        nc.scalar.copy(
                            out=qb[:, j * 1024 : (j + 1) * 1024], in_=psq
                        )
                return qb

            next_q = qprep(0)

            for b in range(BL):
                qb = next_q

                scores = smallp.tile([P, NT], F32, tag="scores")
                tiles = [None] * NT

                # ---- loads: f32 pairs on sync, fp16 casting chunks on Pool
                for ci in range(0, len(F_TS), UCH):
                    t0 = F_TS[ci]
                    fch = fpool.tile([P, UCH, H], F32, tag="Ef", name=f"fch_{b}_{ci}")
                    nc.sync.dma_start(
                        out=fch,
                        in_=enc[b, t0 * P : (t0 + UCH) * P, :].rearrange(
                            "(a p) h -> p a h", p=P
                        ),
                    )
                    for k in range(UCH):
                        tiles[t0 + k] = (fch[:, k, :], False)

                for ci in range(0, len(F16_TS), UCH):
                    grp = F16_TS[ci : ci + UCH]
                    assert grp[1] == grp[0] + 1, grp
                    hch = hpool.tile([P, UCH, H], F16, tag="Eh", name=f"hch_{b}_{ci}")
                    nc.gpsimd.dma_start(
                        out=hch,
                        in_=enc[b, grp[0] * P : (grp[0] + UCH) * P, :].rearrange(
                            "(a p) h -> p a h", p=P
                        ),
                    )
                    for k, t in enumerate(grp):
                        tiles[t] = (hch[:, k, :], True)

                # ---- scores ----
                def score_tile(t):
                    ap, _ = tiles[t]
                    if t in G_TS or t in Y_TS:
                        tmp = tmpp.tile([P, H], F16, tag="tmp")
                        if t in G_TS:
                            nc.gpsimd.tensor_mul(tmp, ap, qb)
                        else:
                            nc.vector.tensor_mul(tmp, ap, qb)
                        nc.scalar.activation(
                            out=tmp, in_=tmp, func=Act.Copy, scale=1.0,
                            accum_out=scores[:, t : t + 1],
                        )
                    else:
                        tmp = tmpp.tile([P, H], F16, tag="tmp")
                        nc.vector.scalar_tensor_tensor(
                            out=tmp, in0=ap, scalar=1.0, in1=qb,
                            op0=Alu.mult, op1=Alu.mult,
                            accum_out=scores[:, t : t + 1],
                        )

                def half_softmax(sl, tag):
                    # returns NEGATIVE max broadcast [P,1]
                    m1 = smallp.tile([P, 1], F32, tag=f"m1{tag}")
                    nc.vector.tensor_reduce(
                        out=m1, in_=scores[:, sl], axis=mybir.AxisListType.X,
                        op=Alu.max,
                    )
                    gm = smallp.tile([1, 1], F32, tag=f"gm{tag}")
                    nc.gpsimd.tensor_reduce(
                        out=gm, in_=m1, axis=mybir.AxisListType.XYZWC, op=Alu.max
                    )
                    pnb = psml.tile([P, 1], F32, tag="pnb")
                    nc.tensor.matmul(
                        pnb, lhsT=neg_ones_row, rhs=gm, start=True, stop=True
                    )
                    nmh = smallp.tile([P, 1], F32, tag=f"nm{tag}")
                    nc.vector.tensor_copy(out=nmh, in_=pnb)
                    return nmh

                def half_exp(sl, neg_m, tag):
                    ecolh = smallp.tile([P, HALF], F16, tag=f"ecolh{tag}")
                    rs = smallp.tile([P, 1], F32, tag=f"rs{tag}")
                    nc.scalar.activation(
                        out=ecolh, in_=scores[:, sl], func=Act.Exp,
                        bias=neg_m, scale=1.0, accum_out=rs,
                    )
                    ecol32 = smallp.tile([P, HALF], F32, tag=f"ecol32{tag}")
                    nc.vector.tensor_copy(out=ecol32, in_=ecolh)
                    return ecolh, ecol32, rs

                def half_ctx(pcx, order, base, ecolh, ecol32):
                    for hb in range(NHB):
                        for i, t in enumerate(order):
                            ap, is16 = tiles[t]
                            nc.tensor.matmul(
                                pcx[:, hb : hb + 1],
                                lhsT=ap[:, hb * P : (hb + 1) * P],
                                rhs=(ecolh if is16 else ecol32)[
                                    :, t - base : t - base + 1
                                ],
                                start=(i == 0),
                                stop=(i == HALF - 1),
                            )

                # A-half scores then softmax
                for t in A_TILES:
                    score_tile(t)
                nmA = half_softmax(slice(0, HALF), "a")
                ecolhA, ecol32A, rsA = half_exp(slice(0, HALF), nmA, "a")

                # B-half scores
                for t in B_TILES:
                    score_tile(t)

                # A context (frees A tiles early); f32 tiles last in chains
                pcxA = pctxa.tile([P, NHB], F32, tag="pcxa", name=f"pcxa_{b}")
                half_ctx(
                    pcxA,
                    [t for t in A_TILES if tiles[t][1]]
                    + [t for t in A_TILES if not tiles[t][1]],
                    0, ecolhA, ecol32A,
                )

                # B softmax with true global max + exact alpha-combine
                nmB = half_softmax(slice(HALF, NT), "b")
                nm = smallp.tile([P, 1], F32, tag="nmg")
                nc.vector.tensor_tensor(out=nm, in0=nmA, in1=nmB, op=Alu.min)
                ecolhB, ecol32B, rsB = half_exp(slice(HALF, NT), nm, "b")

                alphaA = smallp.tile([P, 1], F32, tag="alphaa")
                nc.scalar.activation(
                    out=alphaA, in_=nmA, func=Act.Exp, bias=nm, scale=-1.0
                )
                zrow = smallp.tile([P, 1], F32, tag="zrow")
                nc.vector.scalar_tensor_tensor(
                    out=zrow, in0=rsA, scalar=alphaA, in1=rsB,
                    op0=Alu.mult, op1=Alu.add,
                )
                zg = smallp.tile([1, 1], F32, tag="zg")
                nc.gpsimd.tensor_reduce(
                    out=zg, in_=zrow, axis=mybir.AxisListType.XYZWC, op=Alu.add
                )
                recg = smallp.tile([1, 1], F32, tag="recg")
                nc.vector.reciprocal(recg, zg)
                psr = psml.tile([P, 1], F32, tag="pnb")
                nc.tensor.matmul(
                    psr, lhsT=ones_row32, rhs=recg, start=True, stop=True
                )
                rec_all = smallp.tile([P, 1], F32, tag="recall")
                nc.vector.tensor_copy(out=rec_all, in_=psr)
                scaleA = smallp.tile([P, 1], F32, tag="scalea")
                nc.vector.tensor_mul(scaleA, alphaA, rec_all)

                # B context
                pcxB = pctxb.tile([P, NHB], F32, tag="pcxb", name=f"pcxb_{b}")
                half_ctx(
                    pcxB,
                    [t for t in B_TILES if tiles[t][1]]
                    + [t for t in B_TILES if not tiles[t][1]],
                    HALF, ecolhB, ecol32B,
                )

                # attn out
                attn_sb = smallp.tile([P, NT], F32, tag="attnsb")
                nc.scalar.activation(
                    out=attn_sb[:, 0:HALF], in_=ecolhA, func=Act.Copy,
                    scale=scaleA,
                )
                nc.scalar.activation(
                    out=attn_sb[:, HALF:NT], in_=ecolhB, func=Act.Copy,
                    scale=rec_all,
                )
                nc.scalar.dma_start(out=attn_out[b], in_=attn_sb)

                if b + 1 < BL:
                    next_q = qprep(b + 1)

                flush_deferred()

                ctx_sb = smallp.tile([P, NHB], F32, tag="ctxsb")

                def emit_ctx(
                    pcxA=pcxA, pcxB=pcxB, ctx_sb=ctx_sb,
                    scaleA=scaleA, rec_all=rec_all, b=b,
                ):
                    # one PSUM input per DVE op (walrus NCC_IBVF027)
                    nc.vector.tensor_scalar_mul(
                        out=ctx_sb, in0=pcxB, scalar1=rec_all
                    )
                    nc.vector.scalar_tensor_tensor(
                        out=ctx_sb, in0=pcxA, scalar=scaleA, in1=ctx_sb,
                        op0=Alu.mult, op1=Alu.add,
                    )
                    nc.sync.dma_start(out=ctx_out[b], in_=ctx_sb)

                deferred.append(emit_ctx)
                if b == BL - 1:
                    flush_deferred()

    _split_waits(nc)
    return nc


def _split_waits(nc, maxw=1):
    """Walrus accepts at most one semaphore wait per instruction; move extra
    waits onto NoOp carriers inserted just before (same engine)."""
    from concourse import mybir

    nsplit = 0
    for bb in nc.main_func.blocks:
        newlist = []
        for ins in bb.instructions:
            si = ins.sync_info
            if si is not None and si.on_wait and len(si.on_wait) > maxw:
                waits = list(si.on_wait)
                chunks = [waits[i : i + maxw] for i in range(0, len(waits), maxw)]
                for chunk in chunks[:-1]:
                    pre = mybir.InstNoOp(
                        name=f"{ins.name}_wsplit{nsplit}",
                        engine=ins.engine,
                        ins=[],
                        outs=[],
                        sync_info=mybir.SyncInfo(on_wait=chunk, on_update=[]),
                    )
                    nsplit += 1
                    nc.register_instruction(pre, overwrite=True)
                    newlist.append(pre)
                ins.sync_info = mybir.SyncInfo(
                    on_wait=chunks[-1], on_update=list(si.on_update or [])
                )
            newlist.append(ins)
        bb.instructions[:] = newlist
    return nsplit


def get_nc():
    global _NC
    if _NC is None:
        _NC = _build_nc()
    return _NC


def make_in_maps(hidden, encoder_outputs):
    q = np.asarray(hidden, dtype=np.float32).reshape(B, H)
    enc = np.asarray(encoder_outputs, dtype=np.float32)
    in_maps = []
    for i in range(NCORES):
        in_maps.append(
            {
                "hidden": np.ascontiguousarray(q[i * BL : (i + 1) * BL]),
                "enc": np.ascontiguousarray(enc[i * BL : (i + 1) * BL]),
            }
        )
    return in_maps


def assemble(results):
    # ctx_out[b, p, hb] = context[b, hb*128 + p]
    ctx = np.concatenate(
        [r["ctx_out"].transpose(0, 2, 1).reshape(BL, H) for r in results]
    )
    attn = np.concatenate(
        [r["attn_out"].transpose(0, 2, 1).reshape(BL, S) for r in results]
    )
    return ctx.astype(np.float32), attn.astype(np.float32)


def kernel(hidden, encoder_outputs):
    from concourse.bass_utils import run_bass_kernel_spmd

    nc = get_nc()
    in_maps = make_in_maps(hidden, encoder_outputs)
    res = run_bass_kernel_spmd(nc, in_maps, list(range(NCORES))).results
    return assemble(res)
